# revision 51
# baseline (speedup 1.0000x reference)
"""Trainium2 Bass kernel for MineralDepositGCN (3x GCNConv+BN + MLP head).

Strategy (8 NeuronCores, SPMD single program):
  - Shard nodes by source range: core c owns nodes [c*12500, (c+1)*12500),
    padded to 12800 per core (node n -> padded id 12800*(n//12500)+n%12500).
  - Edges assigned to the core owning src: gather of h[src] is device-local.
  - Per layer: project own shard h@W -> local bf16 gather table [12800, 128]
    (cols 0:64 valid, 64:128 junk pad so rows are 256B for dma_gather);
    dma_gather per-edge rows round-robin across 4 SWDGE queues with a
    8-deep buffer rotation (pad slots point at distinct table rows --
    duplicate-row gathers serialize the DMA path, 3x slowdown); one-hot
    (rebuilt per layer on DVE via two bulk broadcast tensor_tensor ops --
    cheaper than reloading a DRAM cache, which starves the gather queues)
    x messages matmuls segment-sum into PSUM per 512-dst supertile; f32
    partials ReduceScatter(add) across cores so each core ends with the
    aggregate for its own node range; bias+relu+BN with AllReduce'd global
    stats (pad rows contribute exactly relu(bias), subtracted in closed
    form).
  - MLP head computed feature-major on each core's shard; host reassembles.
  - Execution: compiled program + jitted shard_map runner cached at module
    level; repeat kernel() calls only re-upload inputs.
"""
import os
import numpy as np
import ml_dtypes

from concourse import bass, bacc, tile, mybir
from concourse import bass_utils
from concourse.bass_interp import get_hw_module

BF16 = mybir.dt.bfloat16
F32 = mybir.dt.float32
I16 = mybir.dt.int16
ALU = mybir.AluOpType
ACTF = mybir.ActivationFunctionType

NCORES = 8
EPS = 1e-5


def _cfg(n_nodes, in_c, hid, ncls):
    shard = n_nodes // NCORES
    npad = ((shard + 511) // 512) * 512
    return dict(
        N=n_nodes, IN_C=in_c, HID=hid, NCLS=ncls,
        SHARD=shard, NPAD=npad,
        NTILES=npad // 128,            # 128-node dst tiles per core
        NST=npad // 512,               # 512-node supertiles per core
        NST_ALL=(npad // 512) * NCORES,
        NTOT=npad * NCORES,
    )


def _preprocess(x, edge_index, edge_attr, cfg):
    """Host-side sharding: returns per-core input dicts + chunk count C."""
    N, SHARD, NPAD = cfg["N"], cfg["SHARD"], cfg["NPAD"]
    src = edge_index[0].astype(np.int64)
    dst = edge_index[1].astype(np.int64)
    ew = np.asarray(edge_attr, dtype=np.float32)

    owner = src // SHARD
    np.minimum(owner, NCORES - 1, out=owner)   # guard (src < N always)
    local_src = src - owner * SHARD
    # padded global dst id
    dstp = (dst // SHARD) * NPAD + (dst % SHARD)
    gtile = dstp // 128                         # global 128-dst tile id
    NT_ALL = cfg["NTILES"] * NCORES

    # per (core, tile) counts -> C
    counts = np.zeros((NCORES, NT_ALL), dtype=np.int64)
    flat = owner * NT_ALL + gtile
    np.add.at(counts.reshape(-1), flat, 1)
    C = int(max(1, -(-counts.max() // 128)))
    SLOT_T = 128 * C
    NSLOT = NT_ALL * SLOT_T
    NCHUNK = NT_ALL * C

    per_core = []
    for c in range(NCORES):
        m = owner == c
        ls = local_src[m].astype(np.int64)
        dp = dstp[m]
        w = ew[m]
        gt = gtile[m]
        order = np.argsort(gt, kind="stable")
        ls, dp, w, gt = ls[order], dp[order], w[order], gt[order]
        cnt = counts[c]
        # slot position: tile base + rank within tile
        starts = np.zeros(NT_ALL, dtype=np.int64)
        starts[1:] = np.cumsum(cnt)[:-1]
        rank = np.arange(ls.shape[0], dtype=np.int64) - starts[gt]
        slot = gt * SLOT_T + rank

        # pad slots must hit DISTINCT table rows: duplicate-row gathers
        # serialize in the DMA path (3x slowdown measured with all pads
        # pointing at row 0). One-hot weight is 0 for pads, so any row
        # is numerically correct.
        g_idx = (np.arange(NSLOT, dtype=np.int64) % NPAD).astype(np.int16)
        o_dst = np.full(NSLOT, 255.0, dtype=np.float32)  # pad -> no match
        o_ew = np.zeros(NSLOT, dtype=np.float32)
        g_idx[slot] = ls.astype(np.int16)
        o_dst[slot] = (dp - gt * 128).astype(np.float32)
        o_ew[slot] = w

        # dma_gather wrapped index layout, per gather group of GS slots
        GS = cfg["GS"]
        ng = NSLOT // GS
        wrapped = g_idx.reshape(ng, GS // 16, 16).transpose(0, 2, 1)  # [ng,16,GS/16]
        wrapped = wrapped.reshape(ng * 16, GS // 16)
        # -> tensor [128, NSLOT//16]: group g occupies cols [g*GS/16,(g+1)*GS/16)
        idx_t = np.zeros((128, NSLOT // 16), dtype=np.int16)
        for g in range(ng):
            blk = wrapped[g * 16:(g + 1) * 16]            # [16, GS/16]
            idx_t[:, g * (GS // 16):(g + 1) * (GS // 16)] = np.tile(blk, (8, 1))

        per_core.append(dict(
            g_idx=idx_t,
            dst_rel=o_dst.reshape(NCHUNK, 128).T.copy(),
            ew_s=o_ew.reshape(NCHUNK, 128).T.copy(),
        ))

    # x transposed + padded, bf16
    for c in range(NCORES):
        xs = np.zeros((cfg["IN_C"], NPAD), dtype=np.float32)
        xs[:, :SHARD] = np.asarray(x[c * SHARD:(c + 1) * SHARD]).T
        per_core[c]["x_t"] = xs.astype(ml_dtypes.bfloat16)
    return per_core, C, NCHUNK, NSLOT


def _build(cfg, C, NCHUNK, NSLOT, GST):
    IN_C, HID, NCLS = cfg["IN_C"], cfg["HID"], cfg["NCLS"]
    NPAD, NTILES, NST, NST_ALL = (cfg["NPAD"], cfg["NTILES"], cfg["NST"],
                                  cfg["NST_ALL"])
    NPT = NPAD // 128
    GS = cfg["GS"]
    NG = NSLOT // GS                     # gather groups (whole layer)
    CPG = GS // 128                      # chunks per group = GST*4*C
    NPADDING = float(NCORES * NPAD - cfg["N"])
    INVN = 1.0 / cfg["N"]

    KQ = int(os.environ.get("KQUEUES", "4"))
    nc = bacc.Bacc("TRN2", target_bir_lowering=False, debug=False,
                   num_devices=NCORES, num_swdge_queues=KQ)

    def din(name, shape, dt):
        return nc.dram_tensor(name, shape, dt, kind="ExternalInput").ap()

    x_t_d = din("x_t", [IN_C, NPAD], BF16)
    gidx_d = din("g_idx", [128, NSLOT // 16], I16)
    dst_d = din("dst_rel", [128, NCHUNK], F32)
    ew_d = din("ew_s", [128, NCHUNK], F32)
    iota_d = din("iota128", [128, 128], BF16)
    cw_d = [din(f"conv_w{l}", [IN_C if l == 0 else HID, HID], BF16)
            for l in range(3)]
    cb_d = [din(f"conv_b{l}", [HID, 1], F32) for l in range(3)]
    bng_d = [din(f"bn_g{l}", [HID, 1], F32) for l in range(3)]
    bnb_d = [din(f"bn_be{l}", [HID, 1], F32) for l in range(3)]
    mw1_d = din("mlp_w1", [HID, 2 * HID], BF16)
    mw2_d = din("mlp_w2", [2 * HID, HID], BF16)
    mw3_d = din("mlp_w3", [HID, NCLS], BF16)
    mb1_d = din("mlp_b1", [2 * HID, 1], F32)
    mb2_d = din("mlp_b2", [HID, 1], F32)
    mb3_d = din("mlp_b3", [NCLS, 1], F32)
    out_d = nc.dram_tensor("out5", [NCLS, NPAD], F32, kind="ExternalOutput").ap()
    dbg = bool(os.environ.get("KERNEL_DEBUG"))
    if dbg:
        dbg_tab = nc.dram_tensor("dbg_tab", [NPAD, 128], BF16,
                                 kind="ExternalOutput").ap()
        dbg_raw = nc.dram_tensor("dbg_raw", [HID, NPAD], F32,
                                 kind="ExternalOutput").ap()
        dbg_h = nc.dram_tensor("dbg_h", [HID, NPAD], F32,
                               kind="ExternalOutput").ap()

    rg = [list(range(NCORES))]
    SKIP_GATHER = bool(os.environ.get("KSKIP_GATHER"))
    SKIP_CC = bool(os.environ.get("KSKIP_CC"))
    SKIP_CHUNKS = bool(os.environ.get("KSKIP_CHUNKS"))
    SKIP_PROJ = bool(os.environ.get("KSKIP_PROJ"))
    OH_BCAST = os.environ.get("KOH", "bcast") == "bcast"
    PDT = BF16 if os.environ.get("KVAR", "") == "bfp" else F32

    with tile.TileContext(nc) as tc:
        NBUF = int(os.environ.get("KNBUF", "8"))
        with tc.tile_pool(name="sb", bufs=1) as sb, \
             tc.tile_pool(name="sb2", bufs=2) as sb2, \
             tc.tile_pool(name="sb4", bufs=6) as sb4, \
             tc.tile_pool(name="sbg", bufs=NBUF) as sbg, \
             tc.tile_pool(name="sbo", bufs=int(os.environ.get("KOB", "4"))) as sbo, \
             tc.tile_pool(name="ps", bufs=2, space="PSUM") as ps, \
             tc.tile_pool(name="ps1", bufs=1, space="PSUM") as ps1, \
             tc.tile_pool(name="dram", bufs=1, space="DRAM") as dram, \
             tc.tile_pool(name="dram2", bufs=2, space="DRAM") as dram2:

            # ---- persistent loads ----
            iota_t = sb.tile([128, 128], BF16, tag="iota")
            nc.sync.dma_start(out=iota_t[:], in_=iota_d[:])
            cw_t = []
            for l in range(3):
                t = sb.tile([IN_C if l == 0 else HID, HID], BF16, tag=f"cw{l}")
                nc.sync.dma_start(out=t[:], in_=cw_d[l][:])
                cw_t.append(t)
            cb_t, bng_t, bnb_t = [], [], []
            for l in range(3):
                tb = sb.tile([HID, 1], F32, tag=f"cb{l}")
                nc.sync.dma_start(out=tb[:], in_=cb_d[l][:])
                cb_t.append(tb)
                tg = sb.tile([HID, 1], F32, tag=f"bng{l}")
                nc.sync.dma_start(out=tg[:], in_=bng_d[l][:])
                bng_t.append(tg)
                te = sb.tile([HID, 1], F32, tag=f"bnb{l}")
                nc.sync.dma_start(out=te[:], in_=bnb_d[l][:])
                bnb_t.append(te)
            mw1_t = sb.tile([HID, 2 * HID], BF16, tag="mw1")
            nc.sync.dma_start(out=mw1_t[:], in_=mw1_d[:])
            mw2_t = sb.tile([2 * HID, HID], BF16, tag="mw2")
            nc.sync.dma_start(out=mw2_t[:], in_=mw2_d[:])
            mw3_t = sb.tile([HID, NCLS], BF16, tag="mw3")
            nc.sync.dma_start(out=mw3_t[:], in_=mw3_d[:])
            mb1_t = sb.tile([2 * HID, 1], F32, tag="mb1")
            nc.sync.dma_start(out=mb1_t[:], in_=mb1_d[:])
            mb2_t = sb.tile([HID, 1], F32, tag="mb2")
            nc.sync.dma_start(out=mb2_t[:], in_=mb2_d[:])
            mb3_t = sb.tile([NCLS, 1], F32, tag="mb3")
            nc.sync.dma_start(out=mb3_t[:], in_=mb3_d[:])

            x_t = sb2.tile([IN_C, NPAD], BF16, tag="hx")
            nc.sync.dma_start(out=x_t[:], in_=x_t_d[:])

            KREPS = int(os.environ.get("KREPS", "1"))
            for rep in range(KREPS):
              h_cur = x_t   # feature-major current activations
              for l in range(3):
                cdim = IN_C if l == 0 else HID
                # ---- projection -> node-major bf16 table ----
                table = dram2.tile([NPAD, 128], BF16, tag="table")
                KB = NPT if NPT < 25 else 25
                assert NPT % KB == 0
                PB = 8 if KB % 8 == 0 else (4 if KB % 4 == 0 else 1)
                for kb in range(0, NPT, KB):
                    stage = sb2.tile([128, KB, 128], BF16, tag="stage")
                    for k0 in range(0, KB, PB):
                        pp = ps.tile([128, PB * HID], F32, tag="proj",
                                     space="PSUM")
                        for j in range(PB):
                            k = k0 + j
                            nc.tensor.matmul(
                                out=pp[:, j * HID:(j + 1) * HID],
                                lhsT=h_cur[:, (kb + k) * 128:(kb + k + 1) * 128],
                                rhs=cw_t[l][:], start=True, stop=True)
                        nc.any.tensor_copy(
                            out=stage[:, k0:k0 + PB, 0:HID],
                            in_=pp[:].rearrange("p (j f) -> p j f", j=PB))
                    tb_view = table[:].rearrange(
                        "(g k p) f -> g p k f", p=128, k=KB)[kb // KB]
                    nc.sync.dma_start(out=tb_view, in_=stage[:])

                # ---- gather + segment-sum into f32 partials ----
                # one-hots rebuilt on DVE every layer (bulk broadcast ops);
                # cheaper than reloading a DRAM cache, which starves the
                # gather queue rotation
                partial = dram.tile([NST_ALL, HID, 512], PDT, tag="partial")
                IBK = int(os.environ.get("KIBK", "4"))
                assert NG % IBK == 0
                IW = GS // 16
                for g in range(NG):
                    if g % IBK == 0:
                        # block-load idx/dst/ew for IBK groups in one DMA
                        # each: per-group loads stall the gather queue on
                        # HWDGE fixed overhead
                        gb = g
                        idx_b = sb2.tile([128, IBK * IW], I16, tag="idxr")
                        nc.scalar.dma_start(
                            out=idx_b[:],
                            in_=gidx_d[:, gb * IW:(gb + IBK) * IW])
                        if not SKIP_CHUNKS:
                            dst_b = sb2.tile([128, IBK * CPG], F32, tag="dstg")
                            nc.scalar.dma_start(
                                out=dst_b[:],
                                in_=dst_d[:, gb * CPG:(gb + IBK) * CPG])
                            ew_b = sb2.tile([128, IBK * CPG], F32, tag="ewg")
                            nc.scalar.dma_start(
                                out=ew_b[:],
                                in_=ew_d[:, gb * CPG:(gb + IBK) * CPG])
                    j = g - gb
                    m_t = sbg.tile([128, CPG, 128], BF16, tag="msg")
                    if not SKIP_GATHER:
                        nc.gpsimd.dma_gather(
                            out_ap=m_t[:], in_ap=table[:],
                            idxs_ap=idx_b[:, j * IW:(j + 1) * IW],
                            num_idxs=GS, num_idxs_reg=GS, elem_size=128,
                            single_packet=False, queue_num=g % KQ)
                    else:
                        nc.vector.memset(m_t[:, 0:1, 0:2], 0.0)
                    ob_t = sbo.tile([128, CPG, 128], BF16, tag="obig")
                    if SKIP_CHUNKS:
                        nc.vector.memset(ob_t[:, 0:1, 0:2], 0.0)
                    else:
                        dst_t = dst_b[:, j * CPG:(j + 1) * CPG]
                        ew_t = ew_b[:, j * CPG:(j + 1) * CPG]
                        if OH_BCAST:
                            nc.vector.tensor_tensor(
                                out=ob_t[:],
                                in0=dst_t.rearrange(
                                    "p (k o) -> p k o", o=1).broadcast_to(
                                        [128, CPG, 128]),
                                in1=iota_t[:].rearrange(
                                    "p (o f) -> p o f", o=1).broadcast_to(
                                        [128, CPG, 128]),
                                op=ALU.is_equal)
                            nc.vector.tensor_tensor(
                                out=ob_t[:], in0=ob_t[:],
                                in1=ew_t.rearrange(
                                    "p (k o) -> p k o", o=1).broadcast_to(
                                        [128, CPG, 128]),
                                op=ALU.mult)
                    pstage = sb2.tile([HID, GST, 512], PDT, tag="pstage")
                    for s in range(GST):
                        st = g * GST + s
                        agg = ps.tile([HID, 512], F32, tag="agg", space="PSUM")
                        for t4 in range(4):
                            gchunk = (st * 4 + t4) * C
                            for ci in range(C):
                                k = gchunk + ci
                                kk = (s * 4 + t4) * C + ci
                                if SKIP_CHUNKS:
                                    continue
                                if not OH_BCAST:
                                    nc.any.tensor_scalar(
                                        out=ob_t[:, kk, :], in0=iota_t[:],
                                        scalar1=dst_t[:, kk:kk + 1],
                                        scalar2=ew_t[:, kk:kk + 1],
                                        op0=ALU.is_equal, op1=ALU.mult)
                                nc.tensor.matmul(
                                    out=agg[:, t4 * 128:(t4 + 1) * 128],
                                    lhsT=m_t[:, kk, 0:HID],
                                    rhs=ob_t[:, kk, :],
                                    start=(ci == 0), stop=(ci == C - 1))
                        if SKIP_CHUNKS:
                            nc.vector.memset(agg[:, 0:2], 0.0)
                        nc.scalar.activation(out=pstage[:, s], in_=agg[:],
                                             func=ACTF.Copy)
                    nc.sync.dma_start(
                        out=partial[g * GST:(g + 1) * GST].rearrange(
                            "s p n -> p s n"),
                        in_=pstage[:])

                # ---- ReduceScatter: each core gets its own node range ----
                rs_out = dram.tile([NST, HID, 512], PDT, tag="rsout")
                if not SKIP_CC:
                    nc.gpsimd.collective_compute(
                        "ReduceScatter", ALU.add, replica_groups=rg,
                        ins=[partial[:]], outs=[rs_out[:]])
                h_raw = sb.tile([HID, NPAD], BF16, tag="hraw")
                nc.gpsimd.dma_start(
                    out=h_raw[:].rearrange("p (s n) -> p s n", s=NST),
                    in_=rs_out[:].rearrange("s p n -> p s n"))
                if dbg and l == 0:
                    nc.sync.dma_start(out=dbg_tab[:], in_=table[:])
                    draw = sb.tile([HID, NPAD], F32, tag="draw")
                    nc.vector.tensor_copy(out=draw[:], in_=h_raw[:])
                    nc.sync.dma_start(out=dbg_raw[:], in_=draw[:])

                # ---- bias + relu (+ sum accum), stats, BN ----
                h_rel = sb.tile([HID, NPAD], BF16, tag="hrel")
                nc.vector.tensor_scalar(
                    out=h_rel[:], in0=h_raw[:], scalar1=cb_t[l][:],
                    scalar2=0.0, op0=ALU.add, op1=ALU.max)
                ssum = sb.tile([HID, 1], F32, tag="ssum")
                nc.scalar.activation(out=h_raw[:], in_=h_rel[:],
                                     func=ACTF.Copy, accum_out=ssum[:])
                ssq = sb.tile([HID, 1], F32, tag="ssq")
                nc.scalar.activation(out=h_raw[:], in_=h_rel[:],
                                     func=ACTF.Square, accum_out=ssq[:])
                stats_in = dram.tile([HID, 2], F32, tag="stin")
                stats_out = dram.tile([HID, 2], F32, tag="stout")
                # pad-row correction: pads contribute relu(bias) each
                pb = sb.tile([HID, 1], F32, tag="pb")
                nc.vector.tensor_scalar(out=pb[:], in0=cb_t[l][:],
                                        scalar1=0.0, scalar2=None,
                                        op0=ALU.max)
                pb2 = sb.tile([HID, 1], F32, tag="pb2")
                nc.vector.tensor_tensor(out=pb2[:], in0=pb[:], in1=pb[:],
                                        op=ALU.mult)
                sc = sb.tile([HID, 2], F32, tag="statsc")
                nc.vector.tensor_scalar(out=sc[:, 0:1], in0=pb[:],
                                        scalar1=-NPADDING, scalar2=None,
                                        op0=ALU.mult)
                nc.vector.tensor_scalar(out=sc[:, 1:2], in0=pb2[:],
                                        scalar1=-NPADDING, scalar2=None,
                                        op0=ALU.mult)
                nc.vector.tensor_tensor(out=sc[:, 0:1], in0=sc[:, 0:1],
                                        in1=ssum[:], op=ALU.add)
                nc.vector.tensor_tensor(out=sc[:, 1:2], in0=sc[:, 1:2],
                                        in1=ssq[:], op=ALU.add)
                nc.sync.dma_start(out=stats_in[:], in_=sc[:])
                if not SKIP_CC:
                    nc.gpsimd.collective_compute(
                        "AllReduce", ALU.add, replica_groups=rg,
                        ins=[stats_in[:]], outs=[stats_out[:]])
                st_sb = sb.tile([HID, 2], F32, tag="stsb")
                nc.sync.dma_start(out=st_sb[:], in_=stats_out[:])
                mt = sb.tile([HID, 1], F32, tag="mt")
                nc.vector.tensor_scalar(out=mt[:], in0=st_sb[:, 0:1],
                                        scalar1=INVN, scalar2=None,
                                        op0=ALU.mult)
                vt = sb.tile([HID, 1], F32, tag="vt")
                nc.vector.tensor_scalar(out=vt[:], in0=st_sb[:, 1:2],
                                        scalar1=INVN, scalar2=None,
                                        op0=ALU.mult)
                msq = sb.tile([HID, 1], F32, tag="msq")
                nc.vector.tensor_tensor(out=msq[:], in0=mt[:], in1=mt[:],
                                        op=ALU.mult)
                nc.vector.tensor_tensor(out=vt[:], in0=vt[:], in1=msq[:],
                                        op=ALU.subtract)
                nc.vector.tensor_scalar(out=vt[:], in0=vt[:], scalar1=EPS,
                                        scalar2=None, op0=ALU.add)
                sqv = sb.tile([HID, 1], F32, tag="sqv")
                nc.scalar.activation(out=sqv[:], in_=vt[:], func=ACTF.Sqrt)
                rstd = sb.tile([HID, 1], F32, tag="rstd")
                nc.vector.reciprocal(out=rstd[:], in_=sqv[:])
                s_t = sb.tile([HID, 1], F32, tag="sT")
                nc.vector.tensor_tensor(out=s_t[:], in0=bng_t[l][:],
                                        in1=rstd[:], op=ALU.mult)
                t_t = sb.tile([HID, 1], F32, tag="tT")
                nc.vector.tensor_tensor(out=t_t[:], in0=mt[:], in1=s_t[:],
                                        op=ALU.mult)
                nc.vector.tensor_tensor(out=t_t[:], in0=bnb_t[l][:],
                                        in1=t_t[:], op=ALU.subtract)
                h_new = sb2.tile([HID, NPAD], BF16, tag="hx")
                nc.vector.tensor_scalar(out=h_new[:], in0=h_rel[:],
                                        scalar1=s_t[:], scalar2=t_t[:],
                                        op0=ALU.mult, op1=ALU.add)
                if dbg and l == 0:
                    dh = sb.tile([HID, NPAD], F32, tag="dh")
                    nc.vector.tensor_copy(out=dh[:], in_=h_new[:])
                    nc.sync.dma_start(out=dbg_h[:], in_=dh[:])
                h_cur = h_new

              # ---- MLP head (feature-major) ----
              for s in range(NST):
                p1 = ps1.tile([2 * HID, 512], F32, tag="mp1", space="PSUM")
                nc.tensor.matmul(out=p1[:], lhsT=mw1_t[:],
                                 rhs=h_cur[:, s * 512:(s + 1) * 512],
                                 start=True, stop=True)
                a1 = sb2.tile([2 * HID, 512], BF16, tag="a1")
                nc.scalar.activation(out=a1[:], in_=p1[:], func=ACTF.Gelu,
                                     bias=mb1_t[:])
                p2 = ps1.tile([HID, 512], F32, tag="mp2", space="PSUM")
                nc.tensor.matmul(out=p2[:], lhsT=mw2_t[:], rhs=a1[:],
                                 start=True, stop=True)
                a2 = sb2.tile([HID, 512], BF16, tag="a2")
                nc.scalar.activation(out=a2[:], in_=p2[:], func=ACTF.Gelu,
                                     bias=mb2_t[:])
                p3 = ps1.tile([NCLS, 512], F32, tag="mp3", space="PSUM")
                nc.tensor.matmul(out=p3[:], lhsT=mw3_t[:], rhs=a2[:],
                                 start=True, stop=True)
                ob = sb2.tile([NCLS, 512], F32, tag="ob")
                nc.vector.tensor_scalar(out=ob[:], in0=p3[:],
                                        scalar1=mb3_t[:], scalar2=None,
                                        op0=ALU.add)
                nc.sync.dma_start(out=out_d[:, s * 512:(s + 1) * 512],
                                  in_=ob[:])
    nc.compile()
    return nc


class _Runner:
    """Reusable SPMD executor: jitted shard_map around the compiled Bass
    module, built once; subsequent calls only device_put fresh inputs (or
    reuse device-resident ones) and execute."""

    def __init__(self, nc):
        import jax
        from jax.sharding import Mesh, PartitionSpec, NamedSharding
        from jax.experimental.shard_map import shard_map
        from concourse.bass2jax import (_bass_exec_p, partition_id_tensor,
                                        install_neuronx_cc_hook)

        install_neuronx_cc_hook()
        self.nc = nc
        pname = nc.partition_id_tensor.name if nc.partition_id_tensor else None
        in_names, out_names, out_avals, zero_outs = [], [], [], []
        for alloc in nc.m.functions[0].allocations:
            if not isinstance(alloc, mybir.MemoryLocationSet):
                continue
            name = alloc.memorylocations[0].name
            if alloc.kind == "ExternalInput":
                if name != pname:
                    in_names.append(name)
            elif alloc.kind == "ExternalOutput":
                shape = tuple(alloc.tensor_shape)
                dtype = mybir.dt.np(alloc.dtype)
                out_avals.append(jax.core.ShapedArray(shape, dtype))
                out_names.append(name)
                zero_outs.append(np.zeros(shape, dtype))
        self.in_names, self.out_names = in_names, out_names
        self.out_avals, self.zero_outs = out_avals, zero_outs
        n_params, n_outs = len(in_names), len(out_avals)
        all_in = list(in_names) + list(out_names)
        if pname is not None:
            all_in.append(pname)

        def _body(*args):
            operands = list(args)
            if pname is not None:
                operands.append(partition_id_tensor())
            return tuple(_bass_exec_p.bind(
                *operands, out_avals=tuple(out_avals),
                in_names=tuple(all_in), out_names=tuple(out_names),
                lowering_input_output_aliases=(),
                sim_require_finite=True, sim_require_nnan=True, nc=nc))

        devices = jax.devices()[:NCORES]
        self.mesh = Mesh(np.asarray(devices), ("core",))
        self.sharding = NamedSharding(self.mesh, PartitionSpec("core"))
        self.fn = jax.jit(
            shard_map(_body, mesh=self.mesh,
                      in_specs=(PartitionSpec("core"),) * (n_params + n_outs),
                      out_specs=(PartitionSpec("core"),) * n_outs,
                      check_rep=False),
            donate_argnums=tuple(range(n_params, n_params + n_outs)),
            keep_unused=True)
        self.dev_in = None
        self._jax = jax

    def put_inputs(self, in_maps):
        jax = self._jax
        concat = [np.concatenate([np.asarray(in_maps[c][nm])
                                  for c in range(NCORES)], axis=0)
                  for nm in self.in_names]
        self.dev_in = [jax.device_put(a, self.sharding) for a in concat]
        for a in self.dev_in:
            a.block_until_ready()

    def _zeros(self):
        jax = self._jax
        zs = [jax.device_put(
            np.zeros((NCORES * z.shape[0], *z.shape[1:]), z.dtype),
            self.sharding) for z in self.zero_outs]
        for z in zs:
            z.block_until_ready()
        return zs

    def execute(self):
        """One timed execution with device-resident inputs. Returns
        (wall_seconds, out_arrays)."""
        import time
        zs = self._zeros()
        t0 = time.time()
        outs = self.fn(*self.dev_in, *zs)
        for o in outs:
            o.block_until_ready()
        return time.time() - t0, outs

    def run(self, in_maps):
        """Full run: upload inputs, execute, fetch per-core outputs."""
        self.put_inputs(in_maps)
        _, outs = self.execute()
        return self.fetch(outs)

    def fetch(self, outs):
        res = []
        for c in range(NCORES):
            res.append({
                name: np.asarray(outs[i]).reshape(
                    NCORES, *self.out_avals[i].shape)[c]
                for i, name in enumerate(self.out_names)})
        return res

    def steady_walls(self, iters=12):
        return [self.execute()[0] for _ in range(iters)]


def _build_null(cfg):
    """Tiny program with the same I/O style; measures the dispatch floor."""
    IN_C, NPAD, NCLS = cfg["IN_C"], cfg["NPAD"], cfg["NCLS"]
    nc = bacc.Bacc("TRN2", target_bir_lowering=False, debug=False,
                   num_devices=NCORES)
    x_d = nc.dram_tensor("x_t", [IN_C, NPAD], BF16, kind="ExternalInput").ap()
    out_d = nc.dram_tensor("out5", [NCLS, NPAD], F32,
                           kind="ExternalOutput").ap()
    with tile.TileContext(nc) as tc:
        with tc.tile_pool(name="sb", bufs=1) as sb:
            t = sb.tile([NCLS, NPAD], BF16, tag="t")
            nc.sync.dma_start(out=t[:], in_=x_d[0:NCLS, :])
            t2 = sb.tile([NCLS, NPAD], F32, tag="t2")
            nc.vector.tensor_copy(out=t2[:], in_=t[:])
            nc.sync.dma_start(out=out_d[:], in_=t2[:])
    nc.compile()
    return nc


_null_runner = None


def null_floor_walls(iters=12):
    """Steady-state walls of the null program (dispatch floor)."""
    global _null_runner
    if _null_runner is None:
        cfg = kernel._last_cfg
        nc = _build_null(cfg)
        nc.m = get_hw_module(nc.m)
        _null_runner = _Runner(nc)
        xin = kernel._last_in_maps[0]["x_t"]
        _null_runner.put_inputs([{"x_t": xin}] * NCORES)
        _null_runner.execute()   # warm up compile+load
    return _null_runner.steady_walls(iters)


def kernel(x, edge_index, edge_attr,
           conv_w0, conv_b0, conv_w1, conv_b1, conv_w2, conv_b2,
           bn_g0, bn_be0, bn_g1, bn_be1, bn_g2, bn_be2,
           mlp_w1, mlp_b1, mlp_w2, mlp_b2, mlp_w3, mlp_b3):
    x = np.asarray(x)
    N, in_c = x.shape
    hid = np.asarray(conv_w0).shape[1]
    ncls = np.asarray(mlp_w3).shape[1]
    cfg = _cfg(N, in_c, hid, ncls)

    # gather group size: GST supertiles per dma_gather
    GST = int(os.environ.get("KGST", "1"))

    # need C before GS; compute counts first via a cheap pre-pass
    per_core, C, NCHUNK, NSLOT = None, None, None, None
    cfg["GS"] = None
    # C depends only on edge distribution
    src = np.asarray(edge_index[0], dtype=np.int64)
    dst = np.asarray(edge_index[1], dtype=np.int64)
    owner = np.minimum(src // cfg["SHARD"], NCORES - 1)
    dstp = (dst // cfg["SHARD"]) * cfg["NPAD"] + (dst % cfg["SHARD"])
    NT_ALL = cfg["NTILES"] * NCORES
    counts = np.zeros(NCORES * NT_ALL, dtype=np.int64)
    np.add.at(counts, owner * NT_ALL + dstp // 128, 1)
    C = int(max(1, -(-counts.max() // 128)))
    cfg["GS"] = GST * 4 * C * 128

    per_core, C2, NCHUNK, NSLOT = _preprocess(x, edge_index, edge_attr, cfg)
    assert C2 == C

    bf = ml_dtypes.bfloat16
    common = dict(
        iota128=np.tile(np.arange(128, dtype=np.float32).astype(bf), (128, 1)),
        conv_w0=np.asarray(conv_w0).astype(bf),
        conv_w1=np.asarray(conv_w1).astype(bf),
        conv_w2=np.asarray(conv_w2).astype(bf),
        conv_b0=np.asarray(conv_b0, dtype=np.float32).reshape(-1, 1),
        conv_b1=np.asarray(conv_b1, dtype=np.float32).reshape(-1, 1),
        conv_b2=np.asarray(conv_b2, dtype=np.float32).reshape(-1, 1),
        bn_g0=np.asarray(bn_g0, dtype=np.float32).reshape(-1, 1),
        bn_g1=np.asarray(bn_g1, dtype=np.float32).reshape(-1, 1),
        bn_g2=np.asarray(bn_g2, dtype=np.float32).reshape(-1, 1),
        bn_be0=np.asarray(bn_be0, dtype=np.float32).reshape(-1, 1),
        bn_be1=np.asarray(bn_be1, dtype=np.float32).reshape(-1, 1),
        bn_be2=np.asarray(bn_be2, dtype=np.float32).reshape(-1, 1),
        mlp_w1=np.asarray(mlp_w1).astype(bf),
        mlp_w2=np.asarray(mlp_w2).astype(bf),
        mlp_w3=np.asarray(mlp_w3).astype(bf),
        mlp_b1=np.asarray(mlp_b1, dtype=np.float32).reshape(-1, 1),
        mlp_b2=np.asarray(mlp_b2, dtype=np.float32).reshape(-1, 1),
        mlp_b3=np.asarray(mlp_b3, dtype=np.float32).reshape(-1, 1),
    )
    in_maps = []
    for c in range(NCORES):
        m = dict(common)
        m["x_t"] = per_core[c]["x_t"]
        m["g_idx"] = per_core[c]["g_idx"]
        m["dst_rel"] = per_core[c]["dst_rel"]
        m["ew_s"] = per_core[c]["ew_s"]
        in_maps.append(m)

    # program structure depends only on (shapes, C): reuse compiled runner
    key = (N, in_c, hid, ncls, C, NCHUNK, NSLOT, GST)
    runner = _RUNNERS.get(key)
    if runner is None:
        nc = _build(cfg, C, NCHUNK, NSLOT, GST)
        nc.m = get_hw_module(nc.m)
        runner = _Runner(nc)
        _RUNNERS[key] = runner
    results = runner.run(in_maps)

    kernel._last_runner = runner
    kernel._last_nc = runner.nc
    kernel._last_in_maps = in_maps
    kernel._last_cfg = cfg
    out = np.empty((N, cfg["NCLS"]), dtype=np.float32)
    SHARD = cfg["SHARD"]
    for c in range(NCORES):
        out[c * SHARD:(c + 1) * SHARD] = results[c]["out5"][:, :SHARD].T
    return out


_RUNNERS = {}



# revision 53
# speedup vs baseline: 1.3348x; 1.3348x over previous
"""Trainium2 Bass kernel for MineralDepositGCN (3x GCNConv+BN + MLP head).

Strategy (8 NeuronCores, SPMD single program):
  - Shard nodes by source range: core c owns nodes [c*12500, (c+1)*12500),
    padded to 12800 per core (node n -> padded id 12800*(n//12500)+n%12500).
  - Edges assigned to the core owning src: gather of h[src] is device-local.
  - Per layer: project own shard h@W -> local bf16 gather table [12800, 128]
    (cols 0:64 valid, 64:128 junk pad so rows are 256B for dma_gather);
    dma_gather per-edge rows round-robin across 4 SWDGE queues with a
    8-deep buffer rotation (pad slots point at distinct table rows --
    duplicate-row gathers serialize the DMA path, 3x slowdown); one-hot
    (rebuilt per layer on DVE via two bulk broadcast tensor_tensor ops --
    cheaper than reloading a DRAM cache, which starves the gather queues)
    x messages matmuls segment-sum into PSUM per 512-dst supertile; f32
    partials ReduceScatter(add) across cores so each core ends with the
    aggregate for its own node range; bias+relu+BN with AllReduce'd global
    stats (pad rows contribute exactly relu(bias), subtracted in closed
    form).
  - MLP head computed feature-major on each core's shard; host reassembles.
  - Execution: compiled program + jitted shard_map runner cached at module
    level; repeat kernel() calls only re-upload inputs.
"""
import os
import numpy as np
import ml_dtypes

from concourse import bass, bacc, tile, mybir
from concourse import bass_utils
from concourse.bass_interp import get_hw_module

BF16 = mybir.dt.bfloat16
F32 = mybir.dt.float32
I16 = mybir.dt.int16
ALU = mybir.AluOpType
ACTF = mybir.ActivationFunctionType

NCORES = 8
EPS = 1e-5


def _cfg(n_nodes, in_c, hid, ncls):
    shard = n_nodes // NCORES
    npad = ((shard + 511) // 512) * 512
    return dict(
        N=n_nodes, IN_C=in_c, HID=hid, NCLS=ncls,
        SHARD=shard, NPAD=npad,
        NTILES=npad // 128,            # 128-node dst tiles per core
        NST=npad // 512,               # 512-node supertiles per core
        NST_ALL=(npad // 512) * NCORES,
        NTOT=npad * NCORES,
    )


def _preprocess(x, edge_index, edge_attr, cfg):
    """Host-side sharding: returns per-core input dicts + chunk count C."""
    N, SHARD, NPAD = cfg["N"], cfg["SHARD"], cfg["NPAD"]
    src = edge_index[0].astype(np.int64)
    dst = edge_index[1].astype(np.int64)
    ew = np.asarray(edge_attr, dtype=np.float32)

    owner = src // SHARD
    np.minimum(owner, NCORES - 1, out=owner)   # guard (src < N always)
    local_src = src - owner * SHARD
    # padded global dst id
    dstp = (dst // SHARD) * NPAD + (dst % SHARD)
    gtile = dstp // 128                         # global 128-dst tile id
    NT_ALL = cfg["NTILES"] * NCORES

    # per (core, tile) counts -> C
    counts = np.zeros((NCORES, NT_ALL), dtype=np.int64)
    flat = owner * NT_ALL + gtile
    np.add.at(counts.reshape(-1), flat, 1)
    C = int(max(1, -(-counts.max() // 128)))
    SLOT_T = 128 * C
    NSLOT = NT_ALL * SLOT_T
    NCHUNK = NT_ALL * C

    per_core = []
    for c in range(NCORES):
        m = owner == c
        ls = local_src[m].astype(np.int64)
        dp = dstp[m]
        w = ew[m]
        gt = gtile[m]
        order = np.argsort(gt, kind="stable")
        ls, dp, w, gt = ls[order], dp[order], w[order], gt[order]
        cnt = counts[c]
        # slot position: tile base + rank within tile
        starts = np.zeros(NT_ALL, dtype=np.int64)
        starts[1:] = np.cumsum(cnt)[:-1]
        rank = np.arange(ls.shape[0], dtype=np.int64) - starts[gt]
        slot = gt * SLOT_T + rank

        # pad slots must hit DISTINCT table rows: duplicate-row gathers
        # serialize in the DMA path (3x slowdown measured with all pads
        # pointing at row 0). One-hot weight is 0 for pads, so any row
        # is numerically correct.
        g_idx = (np.arange(NSLOT, dtype=np.int64) % NPAD).astype(np.int16)
        o_dst = np.full(NSLOT, 255.0, dtype=np.float32)  # pad -> no match
        o_ew = np.zeros(NSLOT, dtype=np.float32)
        g_idx[slot] = ls.astype(np.int16)
        o_dst[slot] = (dp - gt * 128).astype(np.float32)
        o_ew[slot] = w

        # dma_gather wrapped index layout, per gather group of GS slots
        GS = cfg["GS"]
        ng = NSLOT // GS
        wrapped = g_idx.reshape(ng, GS // 16, 16).transpose(0, 2, 1)  # [ng,16,GS/16]
        wrapped = wrapped.reshape(ng * 16, GS // 16)
        # -> tensor [128, NSLOT//16]: group g occupies cols [g*GS/16,(g+1)*GS/16)
        idx_t = np.zeros((128, NSLOT // 16), dtype=np.int16)
        for g in range(ng):
            blk = wrapped[g * 16:(g + 1) * 16]            # [16, GS/16]
            idx_t[:, g * (GS // 16):(g + 1) * (GS // 16)] = np.tile(blk, (8, 1))

        per_core.append(dict(
            g_idx=idx_t,
            dst_rel=o_dst.reshape(NCHUNK, 128).T.copy(),
            ew_s=o_ew.reshape(NCHUNK, 128).T.copy(),
        ))

    # x transposed + padded, bf16
    for c in range(NCORES):
        xs = np.zeros((cfg["IN_C"], NPAD), dtype=np.float32)
        xs[:, :SHARD] = np.asarray(x[c * SHARD:(c + 1) * SHARD]).T
        per_core[c]["x_t"] = xs.astype(ml_dtypes.bfloat16)
    return per_core, C, NCHUNK, NSLOT


def _build(cfg, C, NCHUNK, NSLOT, GST):
    IN_C, HID, NCLS = cfg["IN_C"], cfg["HID"], cfg["NCLS"]
    NPAD, NTILES, NST, NST_ALL = (cfg["NPAD"], cfg["NTILES"], cfg["NST"],
                                  cfg["NST_ALL"])
    NPT = NPAD // 128
    GS = cfg["GS"]
    NG = NSLOT // GS                     # gather groups (whole layer)
    CPG = GS // 128                      # chunks per group = GST*4*C
    NPADDING = float(NCORES * NPAD - cfg["N"])
    INVN = 1.0 / cfg["N"]

    KQ = int(os.environ.get("KQUEUES", "4"))
    nc = bacc.Bacc("TRN2", target_bir_lowering=False, debug=False,
                   num_devices=NCORES, num_swdge_queues=KQ)

    def din(name, shape, dt):
        return nc.dram_tensor(name, shape, dt, kind="ExternalInput").ap()

    x_t_d = din("x_t", [IN_C, NPAD], BF16)
    gidx_d = din("g_idx", [128, NSLOT // 16], I16)
    dst_d = din("dst_rel", [128, NCHUNK], F32)
    ew_d = din("ew_s", [128, NCHUNK], F32)
    iota_d = din("iota128", [128, 128], BF16)
    cw_d = [din(f"conv_w{l}", [IN_C if l == 0 else HID, HID], BF16)
            for l in range(3)]
    cb_d = [din(f"conv_b{l}", [HID, 1], F32) for l in range(3)]
    bng_d = [din(f"bn_g{l}", [HID, 1], F32) for l in range(3)]
    bnb_d = [din(f"bn_be{l}", [HID, 1], F32) for l in range(3)]
    mw1_d = din("mlp_w1", [HID, 2 * HID], BF16)
    mw2_d = din("mlp_w2", [2 * HID, HID], BF16)
    mw3_d = din("mlp_w3", [HID, NCLS], BF16)
    mb1_d = din("mlp_b1", [2 * HID, 1], F32)
    mb2_d = din("mlp_b2", [HID, 1], F32)
    mb3_d = din("mlp_b3", [NCLS, 1], F32)
    out_d = nc.dram_tensor("out5", [NCLS, NPAD], F32, kind="ExternalOutput").ap()
    dbg = bool(os.environ.get("KERNEL_DEBUG"))
    if dbg:
        dbg_tab = nc.dram_tensor("dbg_tab", [NPAD, 128], BF16,
                                 kind="ExternalOutput").ap()
        dbg_raw = nc.dram_tensor("dbg_raw", [HID, NPAD], F32,
                                 kind="ExternalOutput").ap()
        dbg_h = nc.dram_tensor("dbg_h", [HID, NPAD], F32,
                               kind="ExternalOutput").ap()

    rg = [list(range(NCORES))]
    SKIP_GATHER = bool(os.environ.get("KSKIP_GATHER"))
    SKIP_CC = bool(os.environ.get("KSKIP_CC"))
    SKIP_CHUNKS = bool(os.environ.get("KSKIP_CHUNKS"))
    SKIP_PROJ = bool(os.environ.get("KSKIP_PROJ"))
    OH_BCAST = os.environ.get("KOH", "bcast") == "bcast"
    GSPLIT = int(os.environ.get("KGSPLIT", "2"))
    PDT = BF16 if os.environ.get("KVAR", "") == "bfp" else F32

    with tile.TileContext(nc) as tc:
        NBUF = int(os.environ.get("KNBUF", "8"))
        with tc.tile_pool(name="sb", bufs=1) as sb, \
             tc.tile_pool(name="sb2", bufs=2) as sb2, \
             tc.tile_pool(name="sb4", bufs=6) as sb4, \
             tc.tile_pool(name="sbg", bufs=NBUF) as sbg, \
             tc.tile_pool(name="sbo", bufs=int(os.environ.get("KOB", "4"))) as sbo, \
             tc.tile_pool(name="ps", bufs=2, space="PSUM") as ps, \
             tc.tile_pool(name="ps1", bufs=1, space="PSUM") as ps1, \
             tc.tile_pool(name="dram", bufs=1, space="DRAM") as dram, \
             tc.tile_pool(name="dram2", bufs=2, space="DRAM") as dram2:

            # ---- persistent loads ----
            iota_t = sb.tile([128, 128], BF16, tag="iota")
            nc.sync.dma_start(out=iota_t[:], in_=iota_d[:])
            cw_t = []
            for l in range(3):
                t = sb.tile([IN_C if l == 0 else HID, HID], BF16, tag=f"cw{l}")
                nc.sync.dma_start(out=t[:], in_=cw_d[l][:])
                cw_t.append(t)
            cb_t, bng_t, bnb_t = [], [], []
            for l in range(3):
                tb = sb.tile([HID, 1], F32, tag=f"cb{l}")
                nc.sync.dma_start(out=tb[:], in_=cb_d[l][:])
                cb_t.append(tb)
                tg = sb.tile([HID, 1], F32, tag=f"bng{l}")
                nc.sync.dma_start(out=tg[:], in_=bng_d[l][:])
                bng_t.append(tg)
                te = sb.tile([HID, 1], F32, tag=f"bnb{l}")
                nc.sync.dma_start(out=te[:], in_=bnb_d[l][:])
                bnb_t.append(te)
            mw1_t = sb.tile([HID, 2 * HID], BF16, tag="mw1")
            nc.sync.dma_start(out=mw1_t[:], in_=mw1_d[:])
            mw2_t = sb.tile([2 * HID, HID], BF16, tag="mw2")
            nc.sync.dma_start(out=mw2_t[:], in_=mw2_d[:])
            mw3_t = sb.tile([HID, NCLS], BF16, tag="mw3")
            nc.sync.dma_start(out=mw3_t[:], in_=mw3_d[:])
            mb1_t = sb.tile([2 * HID, 1], F32, tag="mb1")
            nc.sync.dma_start(out=mb1_t[:], in_=mb1_d[:])
            mb2_t = sb.tile([HID, 1], F32, tag="mb2")
            nc.sync.dma_start(out=mb2_t[:], in_=mb2_d[:])
            mb3_t = sb.tile([NCLS, 1], F32, tag="mb3")
            nc.sync.dma_start(out=mb3_t[:], in_=mb3_d[:])

            x_t = sb2.tile([IN_C, NPAD], BF16, tag="hx")
            nc.sync.dma_start(out=x_t[:], in_=x_t_d[:])

            KREPS = int(os.environ.get("KREPS", "1"))
            for rep in range(KREPS):
              h_cur = x_t   # feature-major current activations
              for l in range(3):
                cdim = IN_C if l == 0 else HID
                # ---- projection -> node-major bf16 table ----
                table = dram2.tile([NPAD, 128], BF16, tag="table")
                KB = NPT if NPT < 25 else 25
                assert NPT % KB == 0
                PB = 8 if KB % 8 == 0 else (4 if KB % 4 == 0 else 1)
                for kb in range(0, NPT, KB):
                    stage = sb2.tile([128, KB, 128], BF16, tag="stage")
                    for k0 in range(0, KB, PB):
                        pp = ps.tile([128, PB * HID], F32, tag="proj",
                                     space="PSUM")
                        for j in range(PB):
                            k = k0 + j
                            nc.tensor.matmul(
                                out=pp[:, j * HID:(j + 1) * HID],
                                lhsT=h_cur[:, (kb + k) * 128:(kb + k + 1) * 128],
                                rhs=cw_t[l][:], start=True, stop=True)
                        nc.any.tensor_copy(
                            out=stage[:, k0:k0 + PB, 0:HID],
                            in_=pp[:].rearrange("p (j f) -> p j f", j=PB))
                    tb_view = table[:].rearrange(
                        "(g k p) f -> g p k f", p=128, k=KB)[kb // KB]
                    nc.sync.dma_start(out=tb_view, in_=stage[:])

                # ---- gather + segment-sum into f32 partials ----
                # one-hots rebuilt on DVE every layer (bulk broadcast ops);
                # cheaper than reloading a DRAM cache, which starves the
                # gather queue rotation
                partial = dram.tile([NST_ALL, HID, 512], PDT, tag="partial")
                IBK = int(os.environ.get("KIBK", "4"))
                assert NG % IBK == 0
                IW = GS // 16
                for g in range(NG):
                    if g % IBK == 0:
                        # block-load idx/dst/ew for IBK groups in one DMA
                        # each: per-group loads stall the gather queue on
                        # HWDGE fixed overhead
                        gb = g
                        idx_b = sb2.tile([128, IBK * IW], I16, tag="idxr")
                        nc.scalar.dma_start(
                            out=idx_b[:],
                            in_=gidx_d[:, gb * IW:(gb + IBK) * IW])
                        if not SKIP_CHUNKS:
                            dst_b = sb2.tile([128, IBK * CPG], F32, tag="dstg")
                            nc.scalar.dma_start(
                                out=dst_b[:],
                                in_=dst_d[:, gb * CPG:(gb + IBK) * CPG])
                            ew_b = sb2.tile([128, IBK * CPG], F32, tag="ewg")
                            nc.scalar.dma_start(
                                out=ew_b[:],
                                in_=ew_d[:, gb * CPG:(gb + IBK) * CPG])
                    j = g - gb
                    m_t = sbg.tile([128, CPG, 128], BF16, tag="msg")
                    if not SKIP_GATHER:
                        if GSPLIT == 2:
                            # two half-gathers on different queues: halves
                            # per-group gather latency (consumer waits on the
                            # whole group)
                            h2 = GS // 2
                            for hh in range(2):
                                nc.gpsimd.dma_gather(
                                    out_ap=m_t[:, hh * (CPG // 2):
                                               (hh + 1) * (CPG // 2)],
                                    in_ap=table[:],
                                    idxs_ap=idx_b[:, j * IW + hh * (IW // 2):
                                                  j * IW + (hh + 1) * (IW // 2)],
                                    num_idxs=h2, num_idxs_reg=h2,
                                    elem_size=128, single_packet=False,
                                    queue_num=(2 * g + hh) % KQ)
                        else:
                            nc.gpsimd.dma_gather(
                                out_ap=m_t[:], in_ap=table[:],
                                idxs_ap=idx_b[:, j * IW:(j + 1) * IW],
                                num_idxs=GS, num_idxs_reg=GS, elem_size=128,
                                single_packet=False, queue_num=g % KQ)
                    else:
                        nc.vector.memset(m_t[:, 0:1, 0:2], 0.0)
                    ob_t = sbo.tile([128, CPG, 128], BF16, tag="obig")
                    if SKIP_CHUNKS:
                        nc.vector.memset(ob_t[:, 0:1, 0:2], 0.0)
                    else:
                        dst_t = dst_b[:, j * CPG:(j + 1) * CPG]
                        ew_t = ew_b[:, j * CPG:(j + 1) * CPG]
                        if OH_BCAST:
                            nc.vector.tensor_tensor(
                                out=ob_t[:],
                                in0=dst_t.rearrange(
                                    "p (k o) -> p k o", o=1).broadcast_to(
                                        [128, CPG, 128]),
                                in1=iota_t[:].rearrange(
                                    "p (o f) -> p o f", o=1).broadcast_to(
                                        [128, CPG, 128]),
                                op=ALU.is_equal)
                            nc.vector.tensor_tensor(
                                out=ob_t[:], in0=ob_t[:],
                                in1=ew_t.rearrange(
                                    "p (k o) -> p k o", o=1).broadcast_to(
                                        [128, CPG, 128]),
                                op=ALU.mult)
                    pstage = sb2.tile([HID, GST, 512], PDT, tag="pstage")
                    for s in range(GST):
                        st = g * GST + s
                        agg = ps.tile([HID, 512], F32, tag="agg", space="PSUM")
                        for t4 in range(4):
                            gchunk = (st * 4 + t4) * C
                            for ci in range(C):
                                k = gchunk + ci
                                kk = (s * 4 + t4) * C + ci
                                if SKIP_CHUNKS:
                                    continue
                                if not OH_BCAST:
                                    nc.any.tensor_scalar(
                                        out=ob_t[:, kk, :], in0=iota_t[:],
                                        scalar1=dst_t[:, kk:kk + 1],
                                        scalar2=ew_t[:, kk:kk + 1],
                                        op0=ALU.is_equal, op1=ALU.mult)
                                nc.tensor.matmul(
                                    out=agg[:, t4 * 128:(t4 + 1) * 128],
                                    lhsT=m_t[:, kk, 0:HID],
                                    rhs=ob_t[:, kk, :],
                                    start=(ci == 0), stop=(ci == C - 1))
                        if SKIP_CHUNKS:
                            nc.vector.memset(agg[:, 0:2], 0.0)
                        nc.scalar.activation(out=pstage[:, s], in_=agg[:],
                                             func=ACTF.Copy)
                    nc.sync.dma_start(
                        out=partial[g * GST:(g + 1) * GST].rearrange(
                            "s p n -> p s n"),
                        in_=pstage[:])

                # ---- ReduceScatter: each core gets its own node range ----
                rs_out = dram.tile([NST, HID, 512], PDT, tag="rsout")
                if not SKIP_CC:
                    nc.gpsimd.collective_compute(
                        "ReduceScatter", ALU.add, replica_groups=rg,
                        ins=[partial[:]], outs=[rs_out[:]])
                h_raw = sb.tile([HID, NPAD], BF16, tag="hraw")
                nc.gpsimd.dma_start(
                    out=h_raw[:].rearrange("p (s n) -> p s n", s=NST),
                    in_=rs_out[:].rearrange("s p n -> p s n"))
                if dbg and l == 0:
                    nc.sync.dma_start(out=dbg_tab[:], in_=table[:])
                    draw = sb.tile([HID, NPAD], F32, tag="draw")
                    nc.vector.tensor_copy(out=draw[:], in_=h_raw[:])
                    nc.sync.dma_start(out=dbg_raw[:], in_=draw[:])

                # ---- bias + relu (+ sum accum), stats, BN ----
                h_rel = sb.tile([HID, NPAD], BF16, tag="hrel")
                nc.vector.tensor_scalar(
                    out=h_rel[:], in0=h_raw[:], scalar1=cb_t[l][:],
                    scalar2=0.0, op0=ALU.add, op1=ALU.max)
                ssum = sb.tile([HID, 1], F32, tag="ssum")
                nc.scalar.activation(out=h_raw[:], in_=h_rel[:],
                                     func=ACTF.Copy, accum_out=ssum[:])
                ssq = sb.tile([HID, 1], F32, tag="ssq")
                nc.scalar.activation(out=h_raw[:], in_=h_rel[:],
                                     func=ACTF.Square, accum_out=ssq[:])
                stats_in = dram.tile([HID, 2], F32, tag="stin")
                stats_out = dram.tile([HID, 2], F32, tag="stout")
                # pad-row correction: pads contribute relu(bias) each
                pb = sb.tile([HID, 1], F32, tag="pb")
                nc.vector.tensor_scalar(out=pb[:], in0=cb_t[l][:],
                                        scalar1=0.0, scalar2=None,
                                        op0=ALU.max)
                pb2 = sb.tile([HID, 1], F32, tag="pb2")
                nc.vector.tensor_tensor(out=pb2[:], in0=pb[:], in1=pb[:],
                                        op=ALU.mult)
                sc = sb.tile([HID, 2], F32, tag="statsc")
                nc.vector.tensor_scalar(out=sc[:, 0:1], in0=pb[:],
                                        scalar1=-NPADDING, scalar2=None,
                                        op0=ALU.mult)
                nc.vector.tensor_scalar(out=sc[:, 1:2], in0=pb2[:],
                                        scalar1=-NPADDING, scalar2=None,
                                        op0=ALU.mult)
                nc.vector.tensor_tensor(out=sc[:, 0:1], in0=sc[:, 0:1],
                                        in1=ssum[:], op=ALU.add)
                nc.vector.tensor_tensor(out=sc[:, 1:2], in0=sc[:, 1:2],
                                        in1=ssq[:], op=ALU.add)
                nc.sync.dma_start(out=stats_in[:], in_=sc[:])
                if not SKIP_CC:
                    nc.gpsimd.collective_compute(
                        "AllReduce", ALU.add, replica_groups=rg,
                        ins=[stats_in[:]], outs=[stats_out[:]])
                st_sb = sb.tile([HID, 2], F32, tag="stsb")
                nc.sync.dma_start(out=st_sb[:], in_=stats_out[:])
                mt = sb.tile([HID, 1], F32, tag="mt")
                nc.vector.tensor_scalar(out=mt[:], in0=st_sb[:, 0:1],
                                        scalar1=INVN, scalar2=None,
                                        op0=ALU.mult)
                vt = sb.tile([HID, 1], F32, tag="vt")
                nc.vector.tensor_scalar(out=vt[:], in0=st_sb[:, 1:2],
                                        scalar1=INVN, scalar2=None,
                                        op0=ALU.mult)
                msq = sb.tile([HID, 1], F32, tag="msq")
                nc.vector.tensor_tensor(out=msq[:], in0=mt[:], in1=mt[:],
                                        op=ALU.mult)
                nc.vector.tensor_tensor(out=vt[:], in0=vt[:], in1=msq[:],
                                        op=ALU.subtract)
                nc.vector.tensor_scalar(out=vt[:], in0=vt[:], scalar1=EPS,
                                        scalar2=None, op0=ALU.add)
                sqv = sb.tile([HID, 1], F32, tag="sqv")
                nc.scalar.activation(out=sqv[:], in_=vt[:], func=ACTF.Sqrt)
                rstd = sb.tile([HID, 1], F32, tag="rstd")
                nc.vector.reciprocal(out=rstd[:], in_=sqv[:])
                s_t = sb.tile([HID, 1], F32, tag="sT")
                nc.vector.tensor_tensor(out=s_t[:], in0=bng_t[l][:],
                                        in1=rstd[:], op=ALU.mult)
                t_t = sb.tile([HID, 1], F32, tag="tT")
                nc.vector.tensor_tensor(out=t_t[:], in0=mt[:], in1=s_t[:],
                                        op=ALU.mult)
                nc.vector.tensor_tensor(out=t_t[:], in0=bnb_t[l][:],
                                        in1=t_t[:], op=ALU.subtract)
                h_new = sb2.tile([HID, NPAD], BF16, tag="hx")
                nc.vector.tensor_scalar(out=h_new[:], in0=h_rel[:],
                                        scalar1=s_t[:], scalar2=t_t[:],
                                        op0=ALU.mult, op1=ALU.add)
                if dbg and l == 0:
                    dh = sb.tile([HID, NPAD], F32, tag="dh")
                    nc.vector.tensor_copy(out=dh[:], in_=h_new[:])
                    nc.sync.dma_start(out=dbg_h[:], in_=dh[:])
                h_cur = h_new

              # ---- MLP head (feature-major) ----
              for s in range(NST):
                p1 = ps1.tile([2 * HID, 512], F32, tag="mp1", space="PSUM")
                nc.tensor.matmul(out=p1[:], lhsT=mw1_t[:],
                                 rhs=h_cur[:, s * 512:(s + 1) * 512],
                                 start=True, stop=True)
                a1 = sb2.tile([2 * HID, 512], BF16, tag="a1")
                nc.scalar.activation(out=a1[:], in_=p1[:], func=ACTF.Gelu,
                                     bias=mb1_t[:])
                p2 = ps1.tile([HID, 512], F32, tag="mp2", space="PSUM")
                nc.tensor.matmul(out=p2[:], lhsT=mw2_t[:], rhs=a1[:],
                                 start=True, stop=True)
                a2 = sb2.tile([HID, 512], BF16, tag="a2")
                nc.scalar.activation(out=a2[:], in_=p2[:], func=ACTF.Gelu,
                                     bias=mb2_t[:])
                p3 = ps1.tile([NCLS, 512], F32, tag="mp3", space="PSUM")
                nc.tensor.matmul(out=p3[:], lhsT=mw3_t[:], rhs=a2[:],
                                 start=True, stop=True)
                ob = sb2.tile([NCLS, 512], F32, tag="ob")
                nc.vector.tensor_scalar(out=ob[:], in0=p3[:],
                                        scalar1=mb3_t[:], scalar2=None,
                                        op0=ALU.add)
                nc.sync.dma_start(out=out_d[:, s * 512:(s + 1) * 512],
                                  in_=ob[:])
    nc.compile()
    return nc


class _Runner:
    """Reusable SPMD executor: jitted shard_map around the compiled Bass
    module, built once; subsequent calls only device_put fresh inputs (or
    reuse device-resident ones) and execute."""

    def __init__(self, nc):
        import jax
        from jax.sharding import Mesh, PartitionSpec, NamedSharding
        from jax.experimental.shard_map import shard_map
        from concourse.bass2jax import (_bass_exec_p, partition_id_tensor,
                                        install_neuronx_cc_hook)

        install_neuronx_cc_hook()
        self.nc = nc
        pname = nc.partition_id_tensor.name if nc.partition_id_tensor else None
        in_names, out_names, out_avals, zero_outs = [], [], [], []
        for alloc in nc.m.functions[0].allocations:
            if not isinstance(alloc, mybir.MemoryLocationSet):
                continue
            name = alloc.memorylocations[0].name
            if alloc.kind == "ExternalInput":
                if name != pname:
                    in_names.append(name)
            elif alloc.kind == "ExternalOutput":
                shape = tuple(alloc.tensor_shape)
                dtype = mybir.dt.np(alloc.dtype)
                out_avals.append(jax.core.ShapedArray(shape, dtype))
                out_names.append(name)
                zero_outs.append(np.zeros(shape, dtype))
        self.in_names, self.out_names = in_names, out_names
        self.out_avals, self.zero_outs = out_avals, zero_outs
        n_params, n_outs = len(in_names), len(out_avals)
        all_in = list(in_names) + list(out_names)
        if pname is not None:
            all_in.append(pname)

        def _body(*args):
            operands = list(args)
            if pname is not None:
                operands.append(partition_id_tensor())
            return tuple(_bass_exec_p.bind(
                *operands, out_avals=tuple(out_avals),
                in_names=tuple(all_in), out_names=tuple(out_names),
                lowering_input_output_aliases=(),
                sim_require_finite=True, sim_require_nnan=True, nc=nc))

        devices = jax.devices()[:NCORES]
        self.mesh = Mesh(np.asarray(devices), ("core",))
        self.sharding = NamedSharding(self.mesh, PartitionSpec("core"))
        self.fn = jax.jit(
            shard_map(_body, mesh=self.mesh,
                      in_specs=(PartitionSpec("core"),) * (n_params + n_outs),
                      out_specs=(PartitionSpec("core"),) * n_outs,
                      check_rep=False),
            donate_argnums=tuple(range(n_params, n_params + n_outs)),
            keep_unused=True)
        self.dev_in = None
        self._jax = jax

    def put_inputs(self, in_maps):
        jax = self._jax
        concat = [np.concatenate([np.asarray(in_maps[c][nm])
                                  for c in range(NCORES)], axis=0)
                  for nm in self.in_names]
        self.dev_in = [jax.device_put(a, self.sharding) for a in concat]
        for a in self.dev_in:
            a.block_until_ready()

    def _zeros(self):
        jax = self._jax
        zs = [jax.device_put(
            np.zeros((NCORES * z.shape[0], *z.shape[1:]), z.dtype),
            self.sharding) for z in self.zero_outs]
        for z in zs:
            z.block_until_ready()
        return zs

    def execute(self):
        """One timed execution with device-resident inputs. Returns
        (wall_seconds, out_arrays)."""
        import time
        zs = self._zeros()
        t0 = time.time()
        outs = self.fn(*self.dev_in, *zs)
        for o in outs:
            o.block_until_ready()
        return time.time() - t0, outs

    def run(self, in_maps):
        """Full run: upload inputs, execute, fetch per-core outputs."""
        self.put_inputs(in_maps)
        _, outs = self.execute()
        return self.fetch(outs)

    def fetch(self, outs):
        res = []
        for c in range(NCORES):
            res.append({
                name: np.asarray(outs[i]).reshape(
                    NCORES, *self.out_avals[i].shape)[c]
                for i, name in enumerate(self.out_names)})
        return res

    def steady_walls(self, iters=12):
        return [self.execute()[0] for _ in range(iters)]


def _build_null(cfg):
    """Tiny program with the same I/O style; measures the dispatch floor."""
    IN_C, NPAD, NCLS = cfg["IN_C"], cfg["NPAD"], cfg["NCLS"]
    nc = bacc.Bacc("TRN2", target_bir_lowering=False, debug=False,
                   num_devices=NCORES)
    x_d = nc.dram_tensor("x_t", [IN_C, NPAD], BF16, kind="ExternalInput").ap()
    out_d = nc.dram_tensor("out5", [NCLS, NPAD], F32,
                           kind="ExternalOutput").ap()
    with tile.TileContext(nc) as tc:
        with tc.tile_pool(name="sb", bufs=1) as sb:
            t = sb.tile([NCLS, NPAD], BF16, tag="t")
            nc.sync.dma_start(out=t[:], in_=x_d[0:NCLS, :])
            t2 = sb.tile([NCLS, NPAD], F32, tag="t2")
            nc.vector.tensor_copy(out=t2[:], in_=t[:])
            nc.sync.dma_start(out=out_d[:], in_=t2[:])
    nc.compile()
    return nc


_null_runner = None


def null_floor_walls(iters=12):
    """Steady-state walls of the null program (dispatch floor)."""
    global _null_runner
    if _null_runner is None:
        cfg = kernel._last_cfg
        nc = _build_null(cfg)
        nc.m = get_hw_module(nc.m)
        _null_runner = _Runner(nc)
        xin = kernel._last_in_maps[0]["x_t"]
        _null_runner.put_inputs([{"x_t": xin}] * NCORES)
        _null_runner.execute()   # warm up compile+load
    return _null_runner.steady_walls(iters)


def kernel(x, edge_index, edge_attr,
           conv_w0, conv_b0, conv_w1, conv_b1, conv_w2, conv_b2,
           bn_g0, bn_be0, bn_g1, bn_be1, bn_g2, bn_be2,
           mlp_w1, mlp_b1, mlp_w2, mlp_b2, mlp_w3, mlp_b3):
    x = np.asarray(x)
    N, in_c = x.shape
    hid = np.asarray(conv_w0).shape[1]
    ncls = np.asarray(mlp_w3).shape[1]
    cfg = _cfg(N, in_c, hid, ncls)

    # gather group size: GST supertiles per dma_gather
    GST = int(os.environ.get("KGST", "1"))

    # need C before GS; compute counts first via a cheap pre-pass
    per_core, C, NCHUNK, NSLOT = None, None, None, None
    cfg["GS"] = None
    # C depends only on edge distribution
    src = np.asarray(edge_index[0], dtype=np.int64)
    dst = np.asarray(edge_index[1], dtype=np.int64)
    owner = np.minimum(src // cfg["SHARD"], NCORES - 1)
    dstp = (dst // cfg["SHARD"]) * cfg["NPAD"] + (dst % cfg["SHARD"])
    NT_ALL = cfg["NTILES"] * NCORES
    counts = np.zeros(NCORES * NT_ALL, dtype=np.int64)
    np.add.at(counts, owner * NT_ALL + dstp // 128, 1)
    C = int(max(1, -(-counts.max() // 128)))
    cfg["GS"] = GST * 4 * C * 128

    per_core, C2, NCHUNK, NSLOT = _preprocess(x, edge_index, edge_attr, cfg)
    assert C2 == C

    bf = ml_dtypes.bfloat16
    common = dict(
        iota128=np.tile(np.arange(128, dtype=np.float32).astype(bf), (128, 1)),
        conv_w0=np.asarray(conv_w0).astype(bf),
        conv_w1=np.asarray(conv_w1).astype(bf),
        conv_w2=np.asarray(conv_w2).astype(bf),
        conv_b0=np.asarray(conv_b0, dtype=np.float32).reshape(-1, 1),
        conv_b1=np.asarray(conv_b1, dtype=np.float32).reshape(-1, 1),
        conv_b2=np.asarray(conv_b2, dtype=np.float32).reshape(-1, 1),
        bn_g0=np.asarray(bn_g0, dtype=np.float32).reshape(-1, 1),
        bn_g1=np.asarray(bn_g1, dtype=np.float32).reshape(-1, 1),
        bn_g2=np.asarray(bn_g2, dtype=np.float32).reshape(-1, 1),
        bn_be0=np.asarray(bn_be0, dtype=np.float32).reshape(-1, 1),
        bn_be1=np.asarray(bn_be1, dtype=np.float32).reshape(-1, 1),
        bn_be2=np.asarray(bn_be2, dtype=np.float32).reshape(-1, 1),
        mlp_w1=np.asarray(mlp_w1).astype(bf),
        mlp_w2=np.asarray(mlp_w2).astype(bf),
        mlp_w3=np.asarray(mlp_w3).astype(bf),
        mlp_b1=np.asarray(mlp_b1, dtype=np.float32).reshape(-1, 1),
        mlp_b2=np.asarray(mlp_b2, dtype=np.float32).reshape(-1, 1),
        mlp_b3=np.asarray(mlp_b3, dtype=np.float32).reshape(-1, 1),
    )
    in_maps = []
    for c in range(NCORES):
        m = dict(common)
        m["x_t"] = per_core[c]["x_t"]
        m["g_idx"] = per_core[c]["g_idx"]
        m["dst_rel"] = per_core[c]["dst_rel"]
        m["ew_s"] = per_core[c]["ew_s"]
        in_maps.append(m)

    # program structure depends only on (shapes, C): reuse compiled runner
    key = (N, in_c, hid, ncls, C, NCHUNK, NSLOT, GST)
    runner = _RUNNERS.get(key)
    if runner is None:
        nc = _build(cfg, C, NCHUNK, NSLOT, GST)
        nc.m = get_hw_module(nc.m)
        runner = _Runner(nc)
        _RUNNERS[key] = runner
    results = runner.run(in_maps)

    kernel._last_runner = runner
    kernel._last_nc = runner.nc
    kernel._last_in_maps = in_maps
    kernel._last_cfg = cfg
    out = np.empty((N, cfg["NCLS"]), dtype=np.float32)
    SHARD = cfg["SHARD"]
    for c in range(NCORES):
        out[c * SHARD:(c + 1) * SHARD] = results[c]["out5"][:, :SHARD].T
    return out


_RUNNERS = {}

